# revision 27
# baseline (speedup 1.0000x reference)
"""Trainium2 Bass kernel for AnchorMambaPoolingBlockGated.

Reference computation (per batch element b, channel-first x of shape (D, L)):
    xb = x.reshape(D, N, 2)                    # stride-2 blocks
    mu = xb.mean(-1); mx = xb.max(-1)          # (D, N)
    g  = sigmoid(W @ [mu; mx] + b)             # 1x1 conv over channels
    anchors = g*mx + (1-g)*mu
    out[3k]   = anchors[:, k]
    out[3k+1] = x[:, 2k]
    out[3k+2] = x[:, 2k+1]                     # out is (3N, D)  (transposed!)

Algebra used on device (verified against the reference in numpy):
    mu = 0.5*(e + o)      (e = even tokens, o = odd tokens)
    mx = max(e, o)
    z  = Wmu @ mu + Wmx @ mx + b
    g  = sigmoid(z)
    anchors = mu + g*(mx - mu)
The final add (mu + h, h = g*(mx-mu)) is folded into the anchor transpose:
two accumulating PE transposes sum mu.T and h.T directly in PSUM.

Sharding: data-parallel over batch, core i <- batch element i (B == 8 == n_cores).
No cross-core communication.  Gate weights replicated (pre-folded on host).

Perf strategy (target_regime = memory):
  * 16-bit I/O both ways.  x is cast to f16 and de-interleaved on host into
    chunk-major (n_chunks, D, 2, CHUNK_N) so every chunk load is one fully
    contiguous 1 MB DMA.  The output DRAM tensor is f16 (N, 3, D) and is
    upcast on host; DMA traffic drops 40 MB -> 20 MB per core.
  * The gate matmul runs in fp8 (e4m3) with perf_mode=DoubleRow: weights are
    host-quantized (x16 scale to dodge subnormals, undone in the sigmoid's
    input scale), su/d2 are cast to fp8 on the fly.  2x PE throughput.
  * The (L, D) transposed+interleaved output is assembled in SBUF as
    (128, 4, 3, D) f16 tiles -- anchor/even/odd rows interleaved -- so each
    output DMA writes 3 KB contiguous DRAM segments (1.5 MB per chunk).
  * Elementwise work is spread over DVE / Scalar / GpSimd.
"""

import os
import numpy as np
import ml_dtypes

import concourse.bass as bass
import concourse.tile as tile
from concourse import bacc, mybir
from concourse.alu_op_type import AluOpType
from concourse.bass_utils import run_bass_kernel_spmd

B, D, L = 8, 512, 8192
S = 2
N = L // S                # 4096 pooled blocks
LC = N * (S + 1)          # 12288 output rows per batch
P = 128
DC = D // P               # 4 channel chunks
NCORES = 8

CHUNK_L = 1024            # tokens processed per pipeline chunk
CHUNK_N = CHUNK_L // S    # 512 blocks per chunk
NCHUNKS = L // CHUNK_L    # 8
NQ = CHUNK_N // P         # 4 transpose slices per chunk

W_SCALE = 16.0            # host-side fp8 weight scale (undone in sigmoid)

GATE_MODE = os.environ.get("KERNEL_GATE", "dr")   # "dr" (fp8 DoubleRow) | "f16"

_cache = {}


def _build(gate_mode: str):
    f32 = mybir.dt.float32
    f16 = mybir.dt.float16
    f8 = mybir.dt.float8e4

    nc = bacc.Bacc("TRN2", target_bir_lowering=False, debug=False,
                   num_devices=NCORES)

    # chunk-major de-interleaved input: x5[ci, c, s, nn] = x[c, (ci*CN+nn)*2+s]
    x_ext = nc.declare_dram_parameter("x5", [NCHUNKS, D, S, CHUNK_N], f16,
                                      isOutput=False)
    if gate_mode == "dr":
        # wt8[k, g, i, md, m] = W_SCALE * Wt[256g + 128i + k, 128md + m]
        wt_ext = nc.declare_dram_parameter("wt8", [P, 4, 2, DC, P], f8,
                                           isOutput=False)
    else:
        # wt16[k, kc, md, m] = Wt[128kc + k, 128md + m]
        wt_ext = nc.declare_dram_parameter("wt16", [P, 2 * DC, DC, P], f16,
                                           isOutput=False)
    bias_ext = nc.declare_dram_parameter("bias", [D, 1], f32, isOutput=False)
    id_ext = nc.declare_dram_parameter("ident", [P, P], f16, isOutput=False)
    out_ext = nc.declare_dram_parameter("out", [N, 3, D], f16, isOutput=True)

    sig_scale = 1.0 / W_SCALE if gate_mode == "dr" else 1.0

    with tile.TileContext(nc) as tc:
        with (
            tc.tile_pool(name="consts", bufs=1) as p_const,
            tc.tile_pool(name="xin", bufs=3) as p_x,
            tc.tile_pool(name="pool", bufs=3) as p_pool,
            tc.tile_pool(name="gate", bufs=3) as p_gate,
            tc.tile_pool(name="outt", bufs=3) as p_out,
            tc.tile_pool(name="psz", bufs=2, space="PSUM") as p_psz,
            tc.tile_pool(name="pstt", bufs=4, space="PSUM") as p_pstt,
            tc.tile_pool(name="psta", bufs=2, space="PSUM") as p_psta,
        ):
            # --- first x chunk before consts: it heads the DMA queue ---------
            xt0 = p_x.tile([P, DC, S, CHUNK_N], f16, tag="xt", name="xt0")
            nc.sync.dma_start(
                xt0[:], x_ext[0].rearrange("(c p) s n -> p c s n", p=P))

            # --- constants ---------------------------------------------------
            if gate_mode == "dr":
                wt_sb = p_const.tile([P, 4, 2, DC, P], f8)
            else:
                wt_sb = p_const.tile([P, 2 * DC, DC, P], f16)
            nc.sync.dma_start(wt_sb[:], wt_ext[:])
            ident = p_const.tile([P, P], f16)
            nc.sync.dma_start(ident[:], id_ext[:])
            bias_sb = p_const.tile([P, DC, 1], f32)
            nc.sync.dma_start(bias_sb[:],
                              bias_ext.rearrange("(m p) o -> p m o", p=P))

            # Software-pipelined: stage A(i) = load/pool/cast/token-transpose,
            # stage B(i-1) = gate/blend/anchor/store.  The one-chunk skew
            # keeps every engine's in-order stream free of cross-stage stalls.
            state = {}

            def stage_a(ci):
                if ci == 0:
                    xt = xt0
                else:
                    xt = p_x.tile([P, DC, S, CHUNK_N], f16, tag="xt",
                                  name=f"xt{ci}")
                    nc.sync.dma_start(
                        xt[:],
                        x_ext[ci].rearrange("(c p) s n -> p c s n", p=P))
                e = xt[:, :, 0, :]
                o = xt[:, :, 1, :]

                # pooling: mu = 0.5(e+o), mx = max(e,o), am = mx - mu
                su = p_pool.tile([P, DC, CHUNK_N], f16, tag="su",
                                 name=f"su{ci}")
                mu = p_pool.tile([P, DC, CHUNK_N], f16, tag="mu",
                                 name=f"mu{ci}")
                mx = p_pool.tile([P, DC, CHUNK_N], f16, tag="mx",
                                 name=f"mx{ci}")
                am = p_pool.tile([P, DC, CHUNK_N], f16, tag="am",
                                 name=f"am{ci}")
                mu8 = p_pool.tile([P, DC, CHUNK_N], f8, tag="mu8",
                                  name=f"mu8_{ci}")
                mx8 = p_pool.tile([P, DC, CHUNK_N], f8, tag="mx8",
                                  name=f"mx8_{ci}")
                nc.vector.tensor_tensor(su[:], e, o, AluOpType.add)
                nc.vector.tensor_scalar(mu[:], su[:], 0.5, None,
                                        AluOpType.mult)
                nc.vector.tensor_tensor(mx[:], e, o, AluOpType.max)
                nc.vector.tensor_tensor(am[:], mx[:], mu[:],
                                        AluOpType.subtract)
                # fp8 matmul operands: fused halve+cast on DVE, SWDGE for mx
                nc.vector.tensor_scalar(mu8[:], su[:], 0.5, None,
                                        AluOpType.mult)
                nc.gpsimd.dma_start(mx8[:], mx[:])

                # x tokens: transpose + copy early, free PSUM fast
                ot = p_out.tile([P, NQ, 3, D], f16, tag="ot", name=f"ot{ci}")
                for q in range(NQ):
                    c0 = q * P
                    pst = p_pstt.tile([P, 2, D], f16, tag="pst",
                                      name=f"pst{ci}_{q}")
                    for dc in range(DC):
                        nc.tensor.transpose(
                            pst[:, 0, dc * P:(dc + 1) * P],
                            xt[:, dc, 0, c0:c0 + P], ident[:])
                        nc.tensor.transpose(
                            pst[:, 1, dc * P:(dc + 1) * P],
                            xt[:, dc, 1, c0:c0 + P], ident[:])
                    if q % 2 == 0:
                        nc.vector.tensor_copy(ot[:, q, 1:3, :], pst[:])
                    else:
                        nc.scalar.copy(ot[:, q, 1:3, :], pst[:])
                state[ci] = [mu, am, mu8, mx8, ot, None]

            def stage_b1(ci):
                mu, am, mu8, mx8, ot, _ = state[ci]
                h = p_gate.tile([P, DC, CHUNK_N], f16, tag="h",
                                name=f"h{ci}")
                g = p_gate.tile([P, DC, CHUNK_N], f16, tag="g",
                                name=f"g{ci}")
                ar = p_gate.tile([P, DC, CHUNK_N], f16, tag="ar",
                                name=f"ar{ci}")
                for md in range(DC):
                    ps = p_psz.tile([P, CHUNK_N], f32, tag="psz",
                                    name=f"psz{ci}_{md}")
                    if gate_mode == "dr":
                        for gi in range(4):
                            lhsT = wt_sb[:, gi, :, md, :]
                            rhs = (mu8[:, 2 * gi:2 * gi + 2, :] if gi < 2
                                   else mx8[:, 2 * gi - 4:2 * gi - 2, :])
                            nc.tensor.matmul(
                                ps[:], lhsT, rhs,
                                start=(gi == 0), stop=(gi == 3),
                                perf_mode=mybir.MatmulPerfMode.DoubleRow)
                    else:
                        for kc in range(2 * DC):
                            lhsT = wt_sb[:, kc, md, :]
                            rhs = (mu8[:, kc, :] if kc < DC
                                   else mx8[:, kc - DC, :])
                            nc.tensor.matmul(ps[:], lhsT, rhs,
                                             start=(kc == 0),
                                             stop=(kc == 2 * DC - 1))
                    nc.scalar.activation(g[:, md], ps[:],
                                         mybir.ActivationFunctionType.Sigmoid,
                                         bias=bias_sb[:, md, :],
                                         scale=sig_scale)
                    nc.vector.tensor_tensor(h[:, md], g[:, md], am[:, md],
                                            AluOpType.mult)
                    nc.vector.tensor_tensor(ar[:, md], mu[:, md], h[:, md],
                                            AluOpType.add)
                state[ci][5] = ar

            def stage_b2(ci):
                mu, am, mu8, mx8, ot, ar = state.pop(ci)
                n0 = ci * CHUNK_N
                dv = out_ext[n0:n0 + CHUNK_N, :, :].rearrange(
                    "(q p) r d -> p q r d", p=P)
                for q in range(NQ):
                    c0 = q * P
                    psa = p_psta.tile([P, D], f16, tag="psa",
                                      name=f"psa{ci}_{q}")
                    for md in range(DC):
                        nc.tensor.transpose(
                            psa[:, md * P:(md + 1) * P],
                            ar[:, md, c0:c0 + P], ident[:])
                    nc.scalar.copy(ot[:, q, 0, :], psa[:])
                    nc.sync.dma_start(dv[:, q], ot[:, q])

            for ci in range(NCHUNKS + 2):
                if ci < NCHUNKS:
                    stage_a(ci)
                if 1 <= ci < NCHUNKS + 1:
                    stage_b1(ci - 1)
                if ci >= 2:
                    stage_b2(ci - 2)

    nc.compile()
    return nc


def _get_nc(gate_mode=GATE_MODE):
    if gate_mode not in _cache:
        _cache[gate_mode] = _build(gate_mode)
    return _cache[gate_mode]


def _prep_weights(gate_w: np.ndarray, gate_mode: str):
    gw = np.asarray(gate_w, dtype=np.float32)
    w_mu, w_mx = gw[:, :D], gw[:, D:]
    # z = Wmu @ mu + Wmx @ mx
    wt = np.concatenate([w_mu.T, w_mx.T], axis=0)         # (2D, D), wt[c, d]
    if gate_mode == "dr":
        # wt8[k, g, i, md, m] = W_SCALE * wt[256g + 128i + k, 128md + m]
        w = (W_SCALE * wt).reshape(4, 2, P, DC, P).transpose(2, 0, 1, 3, 4)
        return np.ascontiguousarray(w.astype(ml_dtypes.float8_e4m3))
    # wt16[k, kc, md, m] = wt[128kc + k, 128md + m]
    w = wt.reshape(2 * DC, P, DC, P).transpose(1, 0, 2, 3)
    return np.ascontiguousarray(w.astype(np.float16))


LAST_RESULTS = None


def kernel(x, gate_w, gate_b, mask):
    global LAST_RESULTS
    gate_mode = GATE_MODE
    nc = _get_nc(gate_mode)

    x = np.asarray(x, dtype=np.float32)
    wt = _prep_weights(gate_w, gate_mode)
    wt_name = "wt8" if gate_mode == "dr" else "wt16"
    bias = np.ascontiguousarray(np.asarray(gate_b, np.float32).reshape(D, 1))
    ident = np.eye(P, dtype=np.float16)

    in_maps = []
    for b in range(NCORES):
        x5 = (x[b].astype(np.float16)
              .reshape(D, NCHUNKS, CHUNK_N, S)
              .transpose(1, 0, 3, 2))          # (nchunks, D, S, CHUNK_N)
        in_maps.append({"x5": np.ascontiguousarray(x5), wt_name: wt,
                        "bias": bias, "ident": ident})
    res = run_bass_kernel_spmd(nc, in_maps, core_ids=list(range(NCORES)))
    LAST_RESULTS = res
    out = np.stack([res.results[i]["out"].reshape(LC, D)
                    for i in range(NCORES)])
    return out.astype(np.float32)
